# revision 78
# baseline (speedup 1.0000x reference)
"""Trainium2 Bass kernel: 16-head MHA (B=2, T=2048, D=1024, d_k=64).

Sharding (8 NeuronCores): data-parallel over the batch (2) x tensor-parallel
over head groups (4 groups of 4 heads).  Core c handles batch b = c//4 and
heads [4g, 4g+4) with g = c%4.  Each core computes its partial output
    sum_{h in group} softmax((q Wq_h + bq_h)(k Wk_h)^T / 8) (v Wv_h) Wo_h
and the host sums the 4 partials per batch and adds the constant row
bo + bv @ Wo once.  bk is dropped: with the all-ones mask it shifts every
score row by a per-row constant, which softmax ignores exactly.

Numerics: every matmul operand is bf16 (fp32 PSUM accumulation everywhere);
running the PE in bf16 mode instead of fp32-HIGH is the main speed lever
(~2.5x on the attention matmuls).  Softmax denominators accumulate in fp32;
reciprocals via exp(-ln(x)) on the scalar engine, batched over quarter-
aligned partitions of one [128, 1024] tile, then broadcast across
partitions with rank-1 bf16 matmuls.  The partial outputs stream back as
bf16 and the host reduces them in fp32.  End-to-end error stays in the
few-1e-3 relative class.

The kernel is one software-pipelined stream, paced by the scalar engine's
exp (the roofline resource at ~146us busy):
  * Projections for K and Q run first (two 4-bank PSUM half-passes each,
    x^T chunks resident in a 10-deep SBUF ring).  Attention "steps" (one
    head, one 128-wide k tile, 1024-wide q slice: 2 scores matmuls into
    [128,1024] PSUM + 1 exp emitting bf16) start during Q's second
    half-pass and continue through the V projection and the PE-transpose
    of V into 16 [128, 260] "V_ext" tiles (per head 64 V columns plus a
    ones column that yields softmax row sums for free in the attn*V
    matmul).  Their attn*V matmuls are held back until V_ext exists,
    then drained at ~3 matmuls/step, sized so the tensor engine stays
    just under the scalar engine's pace even when the power governor
    halves the PE rate.
  * In steady state each step emits scores+exp and then attn*V matmuls
    deferred by two steps, so the in-order tensor queue always has a
    full step of independent work ahead of every wait on the scalar
    engine -- across head, head-pair and q-slice boundaries too.
  * Finish work (reciprocal + normalize of O^T into head-pair-stacked
    bf16 tiles + output projection + DMA) for q slice 0 is drip-fed into
    the early steps of q slice 1 on dedicated PSUM banks; q slice 1's
    first head pair finishes while its second still streams, so only
    half a slice's finish work remains as the serial tail.
"""

import functools
import os

import ml_dtypes
import numpy as np

import concourse.bass as bass
import concourse.mybir as mybir
import concourse.tile as tile
from concourse import bacc
from concourse.bass_utils import run_bass_kernel_spmd
from concourse.masks import make_identity

F32 = mybir.dt.float32
BF16 = mybir.dt.bfloat16
AFT = mybir.ActivationFunctionType
BF = ml_dtypes.bfloat16

D = 1024          # model dim
T = 2048          # sequence length
B = 2             # batch
HEADS = 16        # total heads
DK = 64           # head dim
NCORES = 8
GH = 4            # heads per core
GD = GH * DK      # 256 projection cols per core
NF = D // 128     # 8 contraction chunks
NKT = T // 128    # 16 k/t tiles
SCALE = 1.0 / np.sqrt(np.float32(DK))  # 1/8

# Results of the last run (for test harness introspection: exec_time_ns etc.)
LAST_RESULTS = None


@functools.lru_cache(maxsize=1)
def _build_program():
    nc = bacc.Bacc("TRN2", target_bir_lowering=False, debug=False,
                   num_devices=NCORES)

    xqT = nc.declare_dram_parameter("xqT", [D, T], BF16, isOutput=False)
    xkT = nc.declare_dram_parameter("xkT", [D, T], BF16, isOutput=False)
    xvT = nc.declare_dram_parameter("xvT", [D, T], BF16, isOutput=False)
    wq = nc.declare_dram_parameter("wq", [128, NF * GD], BF16, isOutput=False)
    wk = nc.declare_dram_parameter("wk", [128, NF * GD], BF16, isOutput=False)
    wv = nc.declare_dram_parameter("wv", [128, NF * GD], BF16, isOutput=False)
    wo = nc.declare_dram_parameter("wo", [2, 128, D], BF16, isOutput=False)
    bqv = nc.declare_dram_parameter("bqv", [128, 2], F32, isOutput=False)
    out = nc.declare_dram_parameter("out", [T, D], BF16, isOutput=True)

    with tile.TileContext(nc) as tc:
        # ---- persistent pools -------------------------------------------
        with (
            tc.tile_pool(name="qk", bufs=4) as qk_pool,
            tc.tile_pool(name="vext", bufs=NKT) as vext_pool,
            tc.tile_pool(name="wop", bufs=2) as wo_pool,
            tc.tile_pool(name="otp", bufs=2) as ot_pool,
            tc.tile_pool(name="const", bufs=1) as const_pool,
        ):
            bqv_sb = const_pool.tile([128, 2], F32, tag="bqv")
            ident = const_pool.tile([128, 128], BF16, tag="ident")
            ones_f32 = const_pool.tile([128, DK], F32, tag="ones32")
            ones_sb = const_pool.tile([1, DK], BF16, tag="ones")

            QT = [qk_pool.tile([128, T], BF16, tag="qk", name=f"qt{m}")
                  for m in range(2)]
            KT = [qk_pool.tile([128, T], BF16, tag="qk", name=f"kt{m}")
                  for m in range(2)]
            VE = [vext_pool.tile([128, GH * (DK + 1)], BF16, tag="vext",
                                 name=f"ve{i}") for i in range(NKT)]
            WO = [wo_pool.tile([128, D], BF16, tag="wop", name=f"wo{m}")
                  for m in range(2)]
            OT = [ot_pool.tile([128, T], BF16, tag="ot", name=f"ot{m}")
                  for m in range(2)]

            # The kernel is one software-pipelined stream.  Projections
            # for K and Q run first (two 4-bank PSUM half-passes each,
            # with the 8 x^T chunks resident in a 10-deep SBUF ring).
            # The V projection's matmuls are then interleaved with the
            # first 16 attention steps' scores+exp (whose attn*V work is
            # held back), so the scalar engine saturates ~25us earlier
            # and the PE never runs a long unbroken streak that trips
            # the power governor.  Each later step emits scores+exp and
            # then drains one or two held attn*V steps, keeping a full
            # step of PE work queued ahead of every scalar-engine wait.
            # The finish work of q slice 0 (reciprocal + normalize +
            # output projection) is drip-fed into the early steps of q
            # slice 1 on dedicated PSUM banks, so only the final q slice
            # pays a serial tail.
            with (
                tc.tile_pool(name="wts", bufs=3) as w_pool,
                tc.tile_pool(name="xt", bufs=10) as xt_pool,
                tc.tile_pool(name="vt", bufs=2) as vt_pool,
                tc.tile_pool(name="ep", bufs=34) as epool,
                tc.tile_pool(name="ubp", bufs=8) as ub_pool,
                tc.tile_pool(name="rsp", bufs=2) as rs_pool,
                tc.tile_pool(name="osbp", bufs=4) as out_pool,
                tc.tile_pool(name="psS", bufs=2,
                             space=bass.MemorySpace.PSUM) as psS,
            ):
                VT = [vt_pool.tile([128, T], BF16, tag="vt", name=f"vt{m}")
                      for m in range(2)]

                # flat attention step stream ----------------------------
                steps = [(qs, hp, hh, kt)
                         for qs in range(2) for hp in range(2)
                         for hh in range(2) for kt in range(NKT)]
                rs_t = {}
                rr_t = {}
                ub = {}
                o_store = {}
                pending = []
                emit_i = [0]

                def emit_step():
                    qs, hp, hh, kt = steps[emit_i[0]]
                    emit_i[0] += 1
                    q0 = qs * 1024
                    lo = hh * DK
                    if hp == 0 and hh == 0 and kt == 0:
                        rs_t[qs] = rs_pool.tile([128, 1024], F32, tag="rs",
                                                name=f"rs{qs}")
                        rr_t[qs] = rs_pool.tile([128, 1024], F32,
                                                tag="rr", name=f"rr{qs}")
                        nc.gpsimd.memset(rs_t[qs][:], 1.0)
                    ss = psS.tile([128, 1024], F32, tag="s")
                    for hf in range(2):
                        nc.tensor.matmul(
                            ss[:, hf * 512:(hf + 1) * 512],
                            KT[hp][lo:lo + DK, kt * 128:(kt + 1) * 128],
                            QT[hp][lo:lo + DK,
                                   q0 + hf * 512:q0 + (hf + 1) * 512],
                            start=True, stop=True)
                    e = epool.tile([128, 1024], BF16, tag="e")
                    nc.scalar.activation(e[:], ss[:], AFT.Exp,
                                         scale=float(SCALE))
                    pending.append([qs, hp, hh, kt, e, 0])

                def flush_mm(psO):
                    # emit ONE attn*V matmul (half a step) from the
                    # backlog head
                    ent = pending[0]
                    qs, hp, hh, kt, e, hf = ent
                    h = hp * 2 + hh
                    if kt == 0 and hf == 0:
                        o_store[(qs, h)] = psO.tile(
                            [65, 1024], F32, tag="o", name=f"o{qs}_{h}")
                    o_ps = o_store[(qs, h)]
                    nc.tensor.matmul(
                        o_ps[:, hf * 512:(hf + 1) * 512],
                        VE[kt][:, h * (DK + 1):(h + 1) * (DK + 1)],
                        e[:, hf * 512:(hf + 1) * 512],
                        start=(kt == 0), stop=(kt == NKT - 1))
                    if hf == 1:
                        pending.pop(0)
                        if kt == NKT - 1:
                            drains(hp, hh, o_ps, qs)
                    else:
                        ent[5] = 1

                def drains(hp, hh, o_ps, qs):
                    for hf in range(2):
                        i8 = hp * 4 + hh * 2 + hf
                        u = ub_pool.tile([DK, 512], BF16, tag="ub",
                                         name=f"ub{qs}_{i8}")
                        nc.vector.tensor_copy(
                            u[:], o_ps[0:DK, hf * 512:(hf + 1) * 512])
                        p8 = 32 * (hh * 2 + hf)
                        nc.vector.tensor_copy(
                            rs_t[qs][p8:p8 + 1, hp * 512:(hp + 1) * 512],
                            o_ps[DK:DK + 1, hf * 512:(hf + 1) * 512])
                        ub[(qs, i8)] = u

                def recip(qs, hp=None):
                    # batched 1/x on the vector engine, keeping the
                    # scalar engine free for exp; hp selects one column
                    # half (head pair).  The rp staging copies downcast
                    # the f32 reciprocals to bf16 for the broadcast.
                    sl = (slice(0, 1024) if hp is None
                          else slice(hp * 512, (hp + 1) * 512))
                    nc.vector.reciprocal(rr_t[qs][:, sl], rs_t[qs][:, sl])

                def normalize(qs, i8):
                    q0 = qs * 1024
                    hp, hh, hf = i8 // 4, (i8 // 2) % 2, i8 % 2
                    m, lo = hp, hh * DK
                    p8 = 32 * (hh * 2 + hf)
                    # stage the reciprocal row onto partition 0 for the
                    # rank-1 broadcast matmul
                    rp = rs_pool.tile([1, 512], BF16, tag="rp",
                                      name=f"rp{qs}_{i8}")
                    nc.vector.tensor_copy(
                        rp[:], rr_t[qs][p8:p8 + 1, hp * 512:(hp + 1) * 512])
                    r_ps = psR_h[0].tile([DK, 512], F32, tag="rf",
                                         name=f"rps{qs}_{i8}")
                    nc.tensor.matmul(r_ps[:], ones_sb[:], rp[:],
                                     start=True, stop=True)
                    # odd heads land on partitions 64:128 of the
                    # head-pair-stacked O^T tile via the DVE write base
                    nc.vector.tensor_mul(
                        OT[m][lo:lo + DK, q0 + hf * 512:q0 + (hf + 1) * 512],
                        ub[(qs, i8)][:], r_ps[:])

                def outproj(tt, wide=False):
                    osb = out_pool.tile([128, 1024], BF16, tag="osb")
                    for ei in range(2):
                        # during the final tail the attention pools are
                        # idle: alternate PSUM slots with the psS ring to
                        # keep four output tiles in flight
                        if wide and ei == 1:
                            f_ps = psS.tile([128, 512], F32, tag="s",
                                            name=f"fps{tt}_{ei}")
                        else:
                            f_ps = psR_h[0].tile([128, 512], F32, tag="rf",
                                                 name=f"fps{tt}_{ei}")
                        for m in range(2):
                            nc.tensor.matmul(
                                f_ps[:],
                                OT[m][:, tt * 128:(tt + 1) * 128],
                                WO[m][:, ei * 512:(ei + 1) * 512],
                                start=(m == 0), stop=(m == 1))
                        nc.vector.tensor_copy(
                            osb[:, ei * 512:(ei + 1) * 512], f_ps[:])
                    nc.sync.dma_start(out[tt * 128:(tt + 1) * 128, :],
                                      osb[:])

                def finish_work(qs, slot, wide=False):
                    # slot 0.. within the next q slice's step stream
                    if slot == 0:
                        recip(qs)
                    elif 1 <= slot <= 4:
                        normalize(qs, 2 * (slot - 1))
                        normalize(qs, 2 * (slot - 1) + 1)
                    elif 5 <= slot <= 12:
                        outproj(qs * 8 + (slot - 5), wide=wide)

                psR_h = [None]

                # ---- projections: K and Q, two 4-bank half-passes each --
                with tc.tile_pool(name="psA", bufs=4,
                                  space=bass.MemorySpace.PSUM) as psA:

                    def q_drain(m, qh, ps):
                        nc.vector.tensor_scalar_add(
                            QT[m][:, qh * 512:(qh + 1) * 512], ps[:],
                            bqv_sb[:, m:m + 1])

                    def k_drain(m, qh, ps):
                        nc.vector.tensor_copy(
                            KT[m][:, qh * 512:(qh + 1) * 512], ps[:])

                    def v_drain(m, qh, ps):
                        nc.vector.tensor_copy(
                            VT[m][:, qh * 512:(qh + 1) * 512], ps[:])

                    # K's weights go first on the wire, then its x^T
                    # chunks; the other weight loads and the bias queue
                    # behind them (needed only ~15us later)
                    w_sbs = {}
                    for _nm in ("wk", "wq", "wv"):
                        w_sbs[_nm] = w_pool.tile([128, NF * GD], BF16,
                                                 tag="w", name=f"wsb_{_nm}")
                    nc.sync.dma_start(w_sbs["wk"][:], wk[:])

                    def projection(w_name, x_dram, drain, holds=(0, 0)):
                        w_sb = w_sbs[w_name]
                        xts = []
                        for m in range(2):
                            ps = [psA.tile([128, 512], F32, tag="proj",
                                           name=f"pj{m}_{i}")
                                  for i in range(4)]
                            for fc in range(NF):
                                if m == 0:
                                    xt = xt_pool.tile([128, T], BF16,
                                                      tag="xt")
                                    nc.sync.dma_start(
                                        xt[:],
                                        x_dram[fc * 128:(fc + 1) * 128, :])
                                    xts.append(xt)
                                for qh in range(4):
                                    nc.tensor.matmul(
                                        ps[qh][:],
                                        w_sb[:, fc * GD + m * 128:
                                             fc * GD + (m + 1) * 128],
                                        xts[fc][:, qh * 512:(qh + 1) * 512],
                                        start=(fc == 0),
                                        stop=(fc == NF - 1))
                                for _ in range(holds[m]):
                                    emit_step()
                            for qh in range(4):
                                drain(m, qh, ps[qh])

                    projection("wk", xkT, k_drain)
                    nc.sync.dma_start(w_sbs["wq"][:], wq[:])
                    nc.sync.dma_start(w_sbs["wv"][:], wv[:])
                    nc.sync.dma_start(bqv_sb[:], bqv[:])

                    # constants for the transposes / broadcasts, prepared
                    # on the gpsimd/vector engines while the PE projects
                    make_identity(nc, ident[:])
                    nc.gpsimd.memset(ones_f32[:], 1.0)
                    nc.vector.tensor_copy(ones_sb[:], ones_f32[0:1, :])

                    # heads 0/1 scores only need QT[0]/KT[0]: held
                    # attention steps start during Q's second half-pass
                    # and run through the V projection and the V_ext
                    # transposes, saturating the scalar engine early
                    projection("wq", xqT, q_drain, holds=(0, 1))
                    projection("wv", xvT, v_drain, holds=(1, 1))
                    nc.sync.dma_start(WO[0][:], wo[0])
                    nc.sync.dma_start(WO[1][:], wo[1])

                    # V^T -> V_ext (PE transpose of 128x128 blocks)
                    for kt in range(NKT):
                        ve = VE[kt]
                        ve_r = ve[:].rearrange("p (h x) -> p h x", x=DK + 1)
                        nc.vector.tensor_copy(
                            ve_r[:, :, DK:DK + 1],
                            ones_f32[:, 0:GH].rearrange(
                                "p (h x) -> p h x", x=1))
                        for m in range(2):
                            tp = psA.tile([128, 128], BF16, tag="proj")
                            nc.tensor.transpose(
                                tp[:], VT[m][:, kt * 128:(kt + 1) * 128],
                                ident[:])
                            nc.vector.tensor_copy(
                                ve_r[:, 2 * m:2 * m + 2, 0:DK],
                                tp[:].rearrange("k (h j) -> k h j", j=DK))
                        if kt % 2 == 1:
                            emit_step()

                # ---- attention steps with backlog draining --------------
                with (
                    tc.tile_pool(name="psO", bufs=1,
                                 space=bass.MemorySpace.PSUM) as psO,
                    tc.tile_pool(name="psR", bufs=2,
                                 space=bass.MemorySpace.PSUM) as psR,
                ):
                    psR_h[0] = psR
                    flushed = 0
                    while emit_i[0] < len(steps):
                        qs, hp, hh, kt = steps[emit_i[0]]
                        emit_step()
                        s = emit_i[0] - 1
                        pend_mms = sum(2 - e[5] for e in pending)
                        if s <= 69:
                            # drain the held backlog linearly so all of
                            # q slice 0 is flushed before its finish work
                            target = (128 * (s - 31)) // 38 + 1
                            budget = min(max(0, target - flushed),
                                         pend_mms, 4)
                        else:
                            budget = max(0, pend_mms - 4)
                        for _ in range(budget):
                            flush_mm(psO)
                            flushed += 1
                        if qs == 1:
                            step = (hp * 2 + hh) * NKT + kt
                            if step == 6:
                                assert not any(e[0] == 0 for e in pending)
                            if 6 <= step <= 18:
                                finish_work(0, step - 6)
                            # q slice 1's first head pair finishes while
                            # its second is still streaming
                            elif step == 36:
                                recip(1, hp=0)
                            elif step in (38, 40, 42, 44):
                                normalize(1, (step - 38) // 2)

                    # tail of the final q slice: second head pair only
                    while pending:
                        flush_mm(psO)
                    recip(1, hp=1)
                    for i8 in range(4, 8):
                        normalize(1, i8)
                    for tt in range(8, 16):
                        outproj(tt, wide=True)

    from concourse.bacc import get_activation_tables
    import bass_rust as _br
    _combined = "natural_log_exp_and_others"
    _tabs = []
    for _name, _fns in get_activation_tables(nc.m.arch).items():
        if _name != _combined:
            _fns = _fns - {AFT.Exp, AFT.Ln}
        _tabs.append((_name, _fns))
    _br.insert_act_table_loads(nc, _tabs)
    nc.compile()
    return nc


def _numpy_reference(q, k, v, mask, Wq, bq, Wk, bk, Wv, bv, Wo, bo):
    """Fallback for a non-trivial mask (never hit with the stock inputs)."""
    Bn, Tn, _ = q.shape
    H, dk = HEADS, DK

    def split(x):
        return x.reshape(Bn, Tn, H, dk).transpose(0, 2, 1, 3)

    qh = split(q @ Wq + bq)
    kh = split(k @ Wk + bk)
    vh = split(v @ Wv + bv)
    s = np.einsum("bhqd,bhkd->bhqk", qh, kh) / np.sqrt(np.float32(dk))
    s = np.where(mask, s, -np.inf)
    s = s - s.max(axis=-1, keepdims=True)
    e = np.exp(s)
    a = e / e.sum(axis=-1, keepdims=True)
    o = np.einsum("bhqk,bhkd->bhqd", a, vh)
    o = o.transpose(0, 2, 1, 3).reshape(Bn, Tn, H * dk)
    return (o @ Wo + bo).astype(np.float32)


def kernel(q, k, v, mask, Wq, bq, Wk, bk, Wv, bv, Wo, bo):
    global LAST_RESULTS
    q = np.asarray(q, np.float32)
    k = np.asarray(k, np.float32)
    v = np.asarray(v, np.float32)
    mask = np.asarray(mask, bool)
    Wq, bq = np.asarray(Wq, np.float32), np.asarray(bq, np.float32)
    Wk, bk = np.asarray(Wk, np.float32), np.asarray(bk, np.float32)
    Wv, bv = np.asarray(Wv, np.float32), np.asarray(bv, np.float32)
    Wo, bo = np.asarray(Wo, np.float32), np.asarray(bo, np.float32)

    if not mask.all():
        return _numpy_reference(q, k, v, mask, Wq, bq, Wk, bk, Wv, bv, Wo, bo)

    nc = _build_program()

    # host-side sharding
    xT = {}
    for b in range(B):
        xT[b] = tuple(np.ascontiguousarray(x[b].T.astype(BF))
                      for x in (q, k, v))

    def w_chunks(W, g):
        # (1024, 256) head-group slice -> [128, 8*256] chunk-major layout
        Wg = W[:, g * GD:(g + 1) * GD]
        return np.ascontiguousarray(
            Wg.reshape(NF, 128, GD).transpose(1, 0, 2)
            .reshape(128, NF * GD).astype(BF))

    in_maps = []
    for c in range(NCORES):
        b, g = divmod(c, GH)
        xq_t, xk_t, xv_t = xT[b]
        in_maps.append({
            "xqT": xq_t, "xkT": xk_t, "xvT": xv_t,
            "wq": w_chunks(Wq, g), "wk": w_chunks(Wk, g),
            "wv": w_chunks(Wv, g),
            "wo": np.ascontiguousarray(
                Wo[g * GD:(g + 1) * GD, :].astype(BF)).reshape(2, 128, D),
            "bqv": np.ascontiguousarray(
                bq[g * GD:(g + 1) * GD].reshape(2, 128).T),
        })

    LAST_RESULTS = run_bass_kernel_spmd(
        nc, in_maps, list(range(NCORES)),
        trace=bool(os.environ.get("KERNEL_TRACE")))
    res = LAST_RESULTS.results

    const_row = (bv @ Wo + bo).astype(np.float32)  # attn rows sum to 1
    full = np.empty((B, T, D), np.float32)
    for b in range(B):
        acc = res[b * GH]["out"].astype(np.float32)
        for g in range(1, GH):
            acc = acc + res[b * GH + g]["out"]
        full[b] = acc + const_row
    return full
